# revision 56
# baseline (speedup 1.0000x reference)
"""Trainium2 Bass kernel for 2-layer RNN (nn_RNN_28346784154057).

Shapes: x [32, 2048, 512] f32, hidden [2,32,512] (zeros), w_ih/w_hh [2,512,512],
b_ih/b_hh [2,512], w_out [512,512], b_out [512].  Output: [32, 2048, 512].

Strategy (data-parallel over batch, 8 cores, B_local=4 per core):
  - Hidden state kept TRANSPOSED on device: hT [H(4x128 part-chunks), B].
    Recurrence matmuls use weights as the stationary operand (lhsT = W^T
    blocks, pre-transposed on host), moving operand = hT chunks, so the
    output stays in transposed layout and no per-step transposes are needed.
  - u0[t] = x[t] @ w_ih0^T + b_ih0 + b_hh0 precomputed on device for all t
    (x transposed on-chip via PE-transpose, then big matmuls), stored
    transposed in DRAM as [S, 128, (c,b)].
  - Per-step: psumL0 = u0[t] (identity matmul inject) + Whh0 16-tile stream;
    psumL1 = b1 + Wih1 stream (vs h0[t-1], wavefront) + Whh1 stream;
    tanh on ScalarE -> new hT.  h1T streamed to DRAM.
  - Final projection y = h1 @ w_out^T + b_out batched over 32-step groups.
"""

import sys

for _p in ("/opt/trn_rl_repo",):
    if _p not in sys.path:
        sys.path.insert(0, _p)

import numpy as np

from concourse import bass, bass_utils, bacc, tile, mybir

F32 = mybir.dt.float32
AF = mybir.ActivationFunctionType

B, S, D, H, O, L = 32, 2048, 512, 512, 512, 2
N_CORES = 8
BL = B // N_CORES  # batch per core = 4
CB = 4 * BL  # free width of one hT state = n_chunks * B_local = 16
KC = H // 128  # 4 contraction chunks
PROJ_T = 32  # timesteps per projection group


def _wtile(w_t: np.ndarray) -> np.ndarray:
    """[512k, 512m] (already W^T) -> sbuf layout [128, (kc, m)] = [128, 4*512]."""
    return np.ascontiguousarray(
        w_t.reshape(KC, 128, w_t.shape[1]).transpose(1, 0, 2).reshape(128, -1)
    )


BF16 = mybir.dt.bfloat16


def build_kernel(s_steps: int = S, unroll: int = 8, static_loop: bool = False,
                 reps: int = 1, dbg: bool = False, recur_bf16: bool = False,
                 no_u0_dma: bool = False, skip=(), nchains: int = 1):
    WD = BF16 if recur_bf16 else F32
    nc = bacc.Bacc("TRN2", target_bir_lowering=False, debug=False,
                   num_devices=N_CORES)
    internal_kind = {"kind": "ExternalOutput"} if dbg else {}

    # ---- I/O ----
    x_d = nc.dram_tensor("x", [BL, s_steps, D], F32, kind="ExternalInput").ap()
    wih0_d = nc.dram_tensor("wih0t", [128, KC * H], F32, kind="ExternalInput").ap()
    whh0_d = nc.dram_tensor("whh0t", [128, KC * H], WD, kind="ExternalInput").ap()
    wih1_d = nc.dram_tensor("wih1t", [128, KC * H], WD, kind="ExternalInput").ap()
    whh1_d = nc.dram_tensor("whh1t", [128, KC * H], WD, kind="ExternalInput").ap()
    wout_d = nc.dram_tensor("woutt", [128, KC * O], F32, kind="ExternalInput").ap()
    b0t_d = nc.dram_tensor("b0t", [128, KC], F32, kind="ExternalInput").ap()
    b1bc_d = nc.dram_tensor("b1bc", [128, CB], F32, kind="ExternalInput").ap()
    boutbc_d = nc.dram_tensor("boutbc", [128, O], F32, kind="ExternalInput").ap()
    ident_d = nc.dram_tensor("ident", [128, 128], F32, kind="ExternalInput").ap()
    y_d = nc.dram_tensor("y", [BL, s_steps, O], F32, kind="ExternalOutput").ap()

    # internal DRAM streams (transposed layouts).  h1s has one extra slot:
    # the recurrence runs wavefront-style (layer 1 lags one step), so slot
    # s holds h1[s-1]; slot s_steps is written by the epilogue step.
    u0_d = nc.dram_tensor("u0t", [s_steps, 128, CB], F32, **internal_kind).ap()
    h1s_d = nc.dram_tensor("h1s", [s_steps + unroll, 128, CB], F32,
                           **internal_kind).ap()

    n_bt_tiles = (s_steps * BL) // 128  # groups of 32 timesteps
    t_per_tile = 128 // BL  # 32

    with tile.TileContext(nc) as tc:
        # ---- persistent SBUF: weights + constants ----
        with tc.tile_pool(name="wpool", bufs=1) as wpool, \
             tc.tile_pool(name="cpool", bufs=1) as cpool, \
             tc.tile_pool(name="state", bufs=1) as spool:
            wih0 = wpool.tile([128, KC * H], F32, tag="wih0")
            whh0 = wpool.tile([128, KC * H], WD, tag="whh0")
            wih1 = wpool.tile([128, KC * H], WD, tag="wih1")
            whh1 = wpool.tile([128, KC * H], WD, tag="whh1")
            wout = wpool.tile([128, KC * O], F32, tag="wout")
            b0t = cpool.tile([128, KC], F32, tag="b0t")
            b1bc = cpool.tile([128, CB], F32, tag="b1bc")
            boutbc = cpool.tile([128, O], F32, tag="boutbc")
            ident = cpool.tile([128, 128], F32, tag="ident")
            nc.sync.dma_start(wih0[:], wih0_d)
            nc.sync.dma_start(whh0[:], whh0_d)
            nc.sync.dma_start(wih1[:], wih1_d)
            nc.sync.dma_start(whh1[:], whh1_d)
            nc.sync.dma_start(wout[:], wout_d)
            nc.sync.dma_start(b0t[:], b0t_d)
            nc.sync.dma_start(b1bc[:], b1bc_d)
            nc.sync.dma_start(boutbc[:], boutbc_d)
            nc.sync.dma_start(ident[:], ident_d)

            # persistent state tiles (separate tags -> separate slots)
            h0a = h0b = h1a = h1b = None
            if nchains == 1:
                h0a = spool.tile([128, CB], WD, tag="h0a")
                h0b = spool.tile([128, CB], WD, tag="h0b")
                h1a = spool.tile([128, CB], WD, tag="h1a")
                h1b = spool.tile([128, CB], WD, tag="h1b")
            if "nochain" in skip:
                hz0 = spool.tile([128, CB], WD, tag="hz0")
                hz1 = spool.tile([128, CB], WD, tag="hz1")
                nc.vector.memset(hz0[:], 0.0)
                nc.vector.memset(hz1[:], 0.0)
            # independent per-chain state (batch split into nchains chains)
            bc = BL // nchains
            chst = []
            if nchains > 1:
                for j in range(nchains):
                    tiles = {}
                    for nm in ("h0a", "h0b", "h1a", "h1b"):
                        _cht = spool.tile([128, KC * bc], WD,
                                          tag=f"{nm}c{j}")
                        tiles[nm] = _cht
                    chst.append(tiles)

            def w_block(w, kc, mc):
                return w[:, kc * H + mc * 128:kc * H + (mc + 1) * 128]

            def one_rep():
                # ================= Phase B: u0 precompute =================
                # x viewed so partitions = (t_rel, b): [s,b,d] order
                x_sb = x_d.rearrange("b s d -> s b d")
                u0_p = u0_d.rearrange("s p (c b) -> p c s b", c=KC)
                with tc.tile_pool(name="pb_in", bufs=3) as pin, \
                     tc.tile_pool(name="pb_xt", bufs=3) as pxt, \
                     tc.tile_pool(name="pb_ps", bufs=4, space="PSUM") as pps, \
                     tc.tile_pool(name="pb_out", bufs=3) as pout:
                    for g in range(0 if "phaseB" in skip else n_bt_tiles):
                        t0 = g * t_per_tile
                        xt = pin.tile([128, D], F32, tag="xt")
                        nc.sync.dma_start(xt[:], x_sb[t0:t0 + t_per_tile, :, :])
                        xT = pxt.tile([128, KC * 128], F32, tag="xT")
                        for c in range(KC):
                            pst = pps.tile([128, 128], F32, tag="pst")
                            nc.tensor.transpose(
                                pst[:], xt[:, c * 128:(c + 1) * 128], ident[:])
                            nc.vector.tensor_copy(
                                xT[:, c * 128:(c + 1) * 128], pst[:])
                        u0sb = pout.tile([128, KC * 128], F32, tag="u0sb")
                        for jc in range(KC):
                            psu = pps.tile([128, 128], F32, tag="psu")
                            for kc in range(KC):
                                nc.tensor.matmul(
                                    psu[:], w_block(wih0, kc, jc),
                                    xT[:, kc * 128:(kc + 1) * 128],
                                    start=(kc == 0), stop=(kc == KC - 1))
                            nc.scalar.activation(
                                u0sb[:, jc * 128:(jc + 1) * 128], psu[:],
                                AF.Identity, bias=b0t[:, jc:jc + 1])
                        # u0sb free = (jc, t_rel, b) ; dst p c s b
                        nc.sync.dma_start(
                            u0_p[:, :, t0:t0 + t_per_tile, :], u0sb[:])

                tc.strict_bb_all_engine_barrier()

                # ================= Phase C: recurrence =================
                with tc.tile_pool(name="pc_u0", bufs=4) as pu0, \
                     tc.tile_pool(name="pc_ps", bufs=(2 if nchains > 1 else 4),
                                  space="PSUM") as pcps, \
                     tc.tile_pool(name="pc_st", bufs=2) as pstag:
                    if nchains > 1:
                        for j in range(nchains):
                            nc.vector.memset(chst[j]["h0a"][:], 0.0)
                            nc.vector.memset(chst[j]["h1a"][:], 0.0)
                    else:
                        nc.vector.memset(h0a[:], 0.0)
                        nc.vector.memset(h1a[:], 0.0)

                    h1s_p = h1s_d.rearrange("s p cb -> p s cb")
                    u0_p2 = u0_d.rearrange("s p cb -> p s cb")

                    def step(u0blk, u, stag):
                        cur_h0, prev_h0 = (h0a, h0b) if u % 2 else (h0b, h0a)
                        cur_h1, prev_h1 = (h1a, h1b) if u % 2 else (h1b, h1a)
                        if "nochain" in skip:
                            prev_h0, prev_h1 = hz0, hz1
                        u0t = u0blk[:, u * CB:(u + 1) * CB]
                        dvein = "dveinject" in skip
                        ps0 = pcps.tile([128, CB], F32, tag="ps0")
                        ps1 = pcps.tile([128, CB], F32, tag="ps1")
                        if not dvein:
                            nc.tensor.matmul(ps0[:], ident[:], u0t,
                                             start=True, stop=False)
                            nc.tensor.matmul(ps1[:], ident[:], b1bc[:],
                                             start=True, stop=False)
                        l0_mc = 1 if "l0min" in skip else KC
                        for mc in range(l0_mc):
                            for kc in range(l0_mc):
                                last = (kc == l0_mc - 1)
                                nc.tensor.matmul(
                                    ps0[:, mc * BL:(mc + 1) * BL],
                                    w_block(whh0, kc, mc),
                                    prev_h0[:, kc * BL:(kc + 1) * BL],
                                    start=(dvein and mc == 0 and kc == 0),
                                    stop=last)
                        if dvein:
                            pre0 = pu0.tile([128, CB], F32, tag="pre0")
                            nc.vector.tensor_add(pre0[:], ps0[:], u0t)
                            nc.scalar.activation(cur_h0[:], pre0[:], AF.Tanh)
                        else:
                            nc.scalar.activation(cur_h0[:], ps0[:], AF.Tanh)
                        if "l1" not in skip:
                            for mc in range(KC):
                                for kc in range(KC):
                                    nc.tensor.matmul(
                                        ps1[:, mc * BL:(mc + 1) * BL],
                                        w_block(wih1, kc, mc),
                                        prev_h0[:, kc * BL:(kc + 1) * BL],
                                        start=(dvein and mc == 0 and kc == 0),
                                        stop=False)
                            for mc in range(KC):
                                for kc in range(KC):
                                    last = (kc == KC - 1)
                                    nc.tensor.matmul(
                                        ps1[:, mc * BL:(mc + 1) * BL],
                                        w_block(whh1, kc, mc),
                                        prev_h1[:, kc * BL:(kc + 1) * BL],
                                        start=False, stop=last)
                        if dvein:
                            pre1 = pu0.tile([128, CB], F32, tag="pre1")
                            nc.vector.tensor_add(pre1[:], ps1[:], b1bc[:])
                            nc.scalar.activation(cur_h1[:], pre1[:], AF.Tanh)
                        else:
                            nc.scalar.activation(cur_h1[:], ps1[:], AF.Tanh)
                        if "stag" not in skip:
                            nc.vector.tensor_copy(
                                stag[:, u * CB:(u + 1) * CB], cur_h1[:])

                    def chain_slice(t, c_or_none=None):
                        # AP selecting chain j's (c, b') columns
                        return t

                    def step_chains(u0blk, u, stag):
                        for j in range(nchains):
                            st = chst[j]
                            cur_h0, prev_h0 = ((st["h0a"], st["h0b"])
                                               if u % 2 else
                                               (st["h0b"], st["h0a"]))
                            cur_h1, prev_h1 = ((st["h1a"], st["h1b"])
                                               if u % 2 else
                                               (st["h1b"], st["h1a"]))
                            u0v = u0blk[:].rearrange(
                                "p (t c b) -> p t c b", c=KC, b=BL)[
                                :, u, :, j * bc:(j + 1) * bc]
                            ps0 = pcps.tile([128, KC * bc], F32,
                                            tag=f"ps0c{j}")
                            ps1 = pcps.tile([128, KC * bc], F32,
                                            tag=f"ps1c{j}")
                            for mc in range(KC):
                                for kc in range(KC):
                                    nc.tensor.matmul(
                                        ps0[:, mc * bc:(mc + 1) * bc],
                                        w_block(whh0, kc, mc),
                                        prev_h0[:, kc * bc:(kc + 1) * bc],
                                        start=(mc == 0 and kc == 0),
                                        stop=(kc == KC - 1))
                            pre0 = pu0.tile([128, KC * bc], F32,
                                            tag=f"pre0c{j}")
                            nc.vector.tensor_add(pre0[:], ps0[:], u0v)
                            nc.scalar.activation(cur_h0[:], pre0[:], AF.Tanh)
                            b1v = b1bc[:].rearrange(
                                "p (c b) -> p c b", c=KC)[
                                :, :, j * bc:(j + 1) * bc]
                            for mc in range(KC):
                                for kc in range(KC):
                                    nc.tensor.matmul(
                                        ps1[:, mc * bc:(mc + 1) * bc],
                                        w_block(wih1, kc, mc),
                                        prev_h0[:, kc * bc:(kc + 1) * bc],
                                        start=(mc == 0 and kc == 0),
                                        stop=False)
                            for mc in range(KC):
                                for kc in range(KC):
                                    nc.tensor.matmul(
                                        ps1[:, mc * bc:(mc + 1) * bc],
                                        w_block(whh1, kc, mc),
                                        prev_h1[:, kc * bc:(kc + 1) * bc],
                                        start=False, stop=(kc == KC - 1))
                            pre1 = pu0.tile([128, KC * bc], F32,
                                            tag=f"pre1c{j}")
                            nc.vector.tensor_add(pre1[:], ps1[:], b1v)
                            nc.scalar.activation(cur_h1[:], pre1[:], AF.Tanh)
                            stv = stag[:].rearrange(
                                "p (t c b) -> p t c b", c=KC, b=BL)[
                                :, u, :, j * bc:(j + 1) * bc]
                            nc.vector.tensor_copy(stv, cur_h1[:])

                    the_step = step_chains if nchains > 1 else step

                    if static_loop:
                        for i0 in range(0, s_steps, unroll):
                            u0blk = pu0.tile([128, unroll * CB], F32, tag="u0b")
                            nc.sync.dma_start(
                                u0blk[:], u0_p2[:, i0:i0 + unroll, :])
                            stag = pstag.tile([128, unroll * CB], F32, tag="stg")
                            for u in range(unroll):
                                the_step(u0blk, u, stag)
                            nc.sync.dma_start(
                                h1s_p[:, i0:i0 + unroll, :], stag[:])
                    else:
                        with tc.For_i(0, s_steps, unroll) as iv0:
                            u0blk = pu0.tile([128, unroll * CB], F32, tag="u0b")
                            nc.sync.dma_start(
                                u0blk[:], u0_p2[:, bass.ds(iv0, unroll), :])
                            stag = (None if "stag" in skip else
                                    pstag.tile([128, unroll * CB], F32,
                                               tag="stg"))
                            for u in range(unroll):
                                the_step(u0blk, u, stag)
                            if stag is not None:
                                nc.sync.dma_start(
                                    h1s_p[:, bass.ds(iv0, unroll), :], stag[:])
                    # epilogue: one layer-1-only step producing h1[S-1]
                    # into slot S.  After an even number of steps the
                    # latest h0/h1 live in the "a" tiles.
                    estag = pstag.tile([128, CB], F32, tag="estg")
                    if nchains > 1:
                        for j in range(nchains):
                            st = chst[j]
                            eps = pcps.tile([128, KC * bc], F32,
                                            tag=f"ps1c{j}")
                            for mc in range(KC):
                                for kc in range(KC):
                                    nc.tensor.matmul(
                                        eps[:, mc * bc:(mc + 1) * bc],
                                        w_block(wih1, kc, mc),
                                        st["h0a"][:, kc * bc:(kc + 1) * bc],
                                        start=(mc == 0 and kc == 0),
                                        stop=False)
                            for mc in range(KC):
                                for kc in range(KC):
                                    nc.tensor.matmul(
                                        eps[:, mc * bc:(mc + 1) * bc],
                                        w_block(whh1, kc, mc),
                                        st["h1a"][:, kc * bc:(kc + 1) * bc],
                                        start=False, stop=(kc == KC - 1))
                            epre = pu0.tile([128, KC * bc], F32,
                                            tag=f"pre1c{j}")
                            b1v = b1bc[:].rearrange(
                                "p (c b) -> p c b", c=KC)[
                                :, :, j * bc:(j + 1) * bc]
                            nc.vector.tensor_add(epre[:], eps[:], b1v)
                            ev = estag[:].rearrange(
                                "p (c b) -> p c b", c=KC)[
                                :, :, j * bc:(j + 1) * bc]
                            nc.scalar.activation(ev, epre[:], AF.Tanh)
                    else:
                        eps = pcps.tile([128, CB], F32, tag="ps1")
                        nc.tensor.matmul(eps[:], ident[:], b1bc[:],
                                         start=True, stop=False)
                        for mc in range(KC):
                            for kc in range(KC):
                                nc.tensor.matmul(
                                    eps[:, mc * BL:(mc + 1) * BL],
                                    w_block(wih1, kc, mc),
                                    h0a[:, kc * BL:(kc + 1) * BL],
                                    start=False, stop=False)
                        for mc in range(KC):
                            for kc in range(KC):
                                nc.tensor.matmul(
                                    eps[:, mc * BL:(mc + 1) * BL],
                                    w_block(whh1, kc, mc),
                                    h1a[:, kc * BL:(kc + 1) * BL],
                                    start=False, stop=(kc == KC - 1))
                        nc.scalar.activation(estag[:], eps[:], AF.Tanh)
                    nc.sync.dma_start(h1s_p[:, s_steps:s_steps + 1, :],
                                      estag[:])

                tc.strict_bb_all_engine_barrier()

                # ================= Phase D: projection =================
                y_p = y_d.rearrange("b s o -> s b o")
                h1s_f = h1s_d.rearrange("s p cb -> p s cb")
                with tc.tile_pool(name="pd_in", bufs=3) as pdin, \
                     tc.tile_pool(name="pd_ps", bufs=2, space="PSUM") as pdps, \
                     tc.tile_pool(name="pd_out", bufs=3) as pdout:
                    for g in range(0 if "phaseD" in skip
                                   else s_steps // PROJ_T):
                        t0 = g * PROJ_T
                        h1kcs = []
                        for kc in range(KC):
                            h1kc = pdin.tile([128, PROJ_T * BL], F32,
                                             tag=f"h1kc{kc}")
                            nc.sync.dma_start(
                                h1kc[:],
                                h1s_f[:, t0 + 1:t0 + 1 + PROJ_T,
                                      kc * BL:(kc + 1) * BL])
                            h1kcs.append(h1kc)
                        psy = pdps.tile([128, O], F32, tag="psy")
                        nc.tensor.matmul(psy[:], ident[:], boutbc[:],
                                         start=True, stop=False)
                        for kc in range(KC):
                            nc.tensor.matmul(
                                psy[:], h1kcs[kc][:],
                                wout[:, kc * O:(kc + 1) * O],
                                start=False, stop=(kc == KC - 1))
                        ysb = pdout.tile([128, O], F32, tag="ysb")
                        nc.vector.tensor_copy(ysb[:], psy[:])
                        nc.sync.dma_start(
                            y_p[t0:t0 + PROJ_T, :, :], ysb[:])

            if reps == 1:
                one_rep()
            else:
                with tc.For_i(0, reps, 1):
                    one_rep()

    nc.compile()
    return nc


# ====================================================================
# v2: symmetric 4+4 layer pipeline.
#   cores 0-3 ("L0"): u0 precompute + layer-0 recurrence + p1a batched
#     matmuls, each for one batch quarter (8 rows).
#   cores 4-7 ("L1"): layer-1 recurrence + output projection for the
#     same quarter.  One 4-pair AllGather per chunk ships p1aT across.
# ====================================================================

BL2 = 8          # batch rows per core pair
CB2 = 4 * BL2    # 32: free width of a transposed state [128, (c, b)]
CH = 128         # steps per chunk
TPT2 = 128 // BL2  # 16 timesteps per u0 bt-tile


def build_kernel_v2(s_steps: int = S, unroll: int = 8, reps: int = 1,
                    recur_bf16: bool = True):
    WD = BF16 if recur_bf16 else F32
    nch = s_steps // CH
    nc = bacc.Bacc("TRN2", target_bir_lowering=False, debug=False,
                   num_devices=N_CORES)

    x_d = nc.dram_tensor("x", [BL2, s_steps, D], F32, kind="ExternalInput").ap()
    wih0_d = nc.dram_tensor("wih0t", [128, KC * H], F32, kind="ExternalInput").ap()
    whh0_d = nc.dram_tensor("whh0t", [128, KC * H], WD, kind="ExternalInput").ap()
    wih1_d = nc.dram_tensor("wih1t", [128, KC * H], WD, kind="ExternalInput").ap()
    whh1_d = nc.dram_tensor("whh1t", [128, KC * H], WD, kind="ExternalInput").ap()
    wout_d = nc.dram_tensor("woutt", [128, KC * O], WD, kind="ExternalInput").ap()
    b0t_d = nc.dram_tensor("b0t", [128, KC], F32, kind="ExternalInput").ap()
    b1t_d = nc.dram_tensor("b1t", [128, KC], F32, kind="ExternalInput").ap()
    boutbc_d = nc.dram_tensor("boutbc", [128, O], F32, kind="ExternalInput").ap()
    ident_d = nc.dram_tensor("ident", [128, 128], F32, kind="ExternalInput").ap()
    y_d = nc.dram_tensor("y", [BL2, s_steps, O], F32, kind="ExternalOutput").ap()

    u0_d = nc.dram_tensor("u0loc", [s_steps, 128, CB2], F32).ap()
    h0loc = [nc.dram_tensor(f"h0loc{i}", [CH, 128, CB2], WD).ap()
             for i in range(2)]
    p1aloc = [nc.dram_tensor(f"p1aloc{i}", [CH, 128, CB2], F32).ap()
              for i in range(2)]
    p1adst = [nc.dram_tensor(f"p1adst{i}", [2, CH, 128, CB2], F32).ap()
              for i in range(2)]
    h1loc = [nc.dram_tensor(f"h1loc{i}", [CH, 128, CB2], WD).ap()
             for i in range(2)]

    groups = [[0, 4], [1, 5], [2, 6], [3, 7]]

    with tile.TileContext(nc) as tc:
        pid = nc.partition_id()
        with tc.tile_pool(name="wpool", bufs=1) as wpool, \
             tc.tile_pool(name="cpool", bufs=1) as cpool, \
             tc.tile_pool(name="state", bufs=1) as spool, \
             tc.tile_pool(name="kstag", bufs=1) as kpool:
            wih0 = wpool.tile([128, KC * H], F32, tag="wih0")
            whh0 = wpool.tile([128, KC * H], WD, tag="whh0")
            wih1 = wpool.tile([128, KC * H], WD, tag="wih1")
            whh1 = wpool.tile([128, KC * H], WD, tag="whh1")
            wout = wpool.tile([128, KC * O], WD, tag="wout")
            b0t = cpool.tile([128, KC], F32, tag="b0t")
            b1t = cpool.tile([128, KC], F32, tag="b1t")
            boutbc = cpool.tile([128, O], F32, tag="boutbc")
            ident = cpool.tile([128, 128], F32, tag="ident")
            for t_, d_ in ((wih0, wih0_d), (whh0, whh0_d), (wih1, wih1_d),
                           (whh1, whh1_d), (wout, wout_d), (b0t, b0t_d),
                           (b1t, b1t_d), (boutbc, boutbc_d), (ident, ident_d)):
                nc.sync.dma_start(t_[:], d_)

            h0a = spool.tile([128, CB2], WD, tag="h0a")
            h0b = spool.tile([128, CB2], WD, tag="h0b")
            h1a = spool.tile([128, CB2], WD, tag="h1a")
            h1b = spool.tile([128, CB2], WD, tag="h1b")
            nc.vector.memset(h0a[:], 0.0)
            nc.vector.memset(h1a[:], 0.0)
            # per-kc repack tiles for p1a (L0) / proj (L1)
            kcs = [kpool.tile([128, CH * BL2], WD, tag=f"kc{i}")
                   for i in range(KC)]

            def w_block(w, kc, mc):
                return w[:, kc * H + mc * 128:kc * H + (mc + 1) * 128]

            x_sb = x_d.rearrange("b s d -> s b d")
            u0_p = u0_d.rearrange("s p (c b) -> p c s b", c=KC)

            # ---------- L0 prephase: u0 for all steps ----------
            with tc.If(pid < 4) as _cmp0:
                with tc.tile_pool(name="pb_in", bufs=3) as pin, \
                     tc.tile_pool(name="pb_ps", bufs=4, space="PSUM") as pps, \
                     tc.tile_pool(name="pb_out", bufs=3) as pout:
                    def u0_tile(gv):
                        t0v = nc.snap(gv * TPT2)
                        xt = pin.tile([128, D], F32, tag="xt")
                        nc.sync.dma_start(
                            xt[:], x_sb[bass.ds(t0v, TPT2), :, :])
                        xT = pin.tile([128, KC * 128], F32, tag="xT")
                        for c in range(KC):
                            pst = pps.tile([128, 128], F32, tag="pst")
                            nc.tensor.transpose(
                                pst[:], xt[:, c * 128:(c + 1) * 128], ident[:])
                            nc.vector.tensor_copy(
                                xT[:, c * 128:(c + 1) * 128], pst[:])
                        u0sb = pout.tile([128, KC * 128], F32, tag="u0sb")
                        for jc in range(KC):
                            psu = pps.tile([128, 128], F32, tag="psu")
                            for kc in range(KC):
                                nc.tensor.matmul(
                                    psu[:], w_block(wih0, kc, jc),
                                    xT[:, kc * 128:(kc + 1) * 128],
                                    start=(kc == 0), stop=(kc == KC - 1))
                            nc.scalar.activation(
                                u0sb[:, jc * 128:(jc + 1) * 128], psu[:],
                                AF.Identity, bias=b0t[:, jc:jc + 1])
                        nc.sync.dma_start(
                            u0_p[:, :, bass.ds(t0v, TPT2), :], u0sb[:])
                    with tc.For_i(0, s_steps // TPT2, 1) as gv:
                        u0_tile(gv)

            # ---------- chunk pipeline ----------
            with tc.tile_pool(name="rc_in", bufs=4) as prin, \
                 tc.tile_pool(name="rc_ps", bufs=4, space="PSUM") as prps, \
                 tc.tile_pool(name="rc_big", bufs=2, space="PSUM") as prbig, \
                 tc.tile_pool(name="rc_st", bufs=2) as prst, \
                 tc.tile_pool(name="rc_out", bufs=3) as prout:

                def recur_step(injblk, u, stag, layer):
                    if layer == 0:
                        cur, prev = (h0a, h0b) if u % 2 else (h0b, h0a)
                        w = whh0
                    else:
                        cur, prev = (h1a, h1b) if u % 2 else (h1b, h1a)
                        w = whh1
                    inj = injblk[:, u * CB2:(u + 1) * CB2]
                    ps = prps.tile([128, CB2], F32, tag="ps")
                    nc.tensor.matmul(ps[:], ident[:], inj,
                                     start=True, stop=False)
                    for mc in range(KC):
                        for kc in range(KC):
                            nc.tensor.matmul(
                                ps[:, mc * BL2:(mc + 1) * BL2],
                                w_block(w, kc, mc),
                                prev[:, kc * BL2:(kc + 1) * BL2],
                                start=False, stop=(kc == KC - 1))
                    nc.scalar.activation(cur[:], ps[:], AF.Tanh)
                    nc.vector.tensor_copy(
                        stag[:, u * CB2:(u + 1) * CB2], cur[:])

                def recur_chunk(k, layer, dst_loc):
                    loc_p = dst_loc.rearrange("s p cb -> p s cb")
                    if layer == 0:
                        src_p = u0_d.rearrange("s p cb -> p s cb")
                        src_off = k * CH
                    else:
                        src_p = p1adst[k % 2][0].rearrange("s p cb -> p s cb")
                        src_off = 0
                    with tc.For_i(0, CH, unroll) as iv0:
                        injblk = prin.tile([128, unroll * CB2], F32, tag="injb")
                        if src_off:
                            ivs = nc.snap(iv0 + src_off)
                        else:
                            ivs = iv0
                        nc.sync.dma_start(
                            injblk[:], src_p[:, bass.ds(ivs, unroll), :])
                        stag = prst.tile([128, unroll * CB2], WD, tag="stg")
                        for u in range(unroll):
                            recur_step(injblk, u, stag, layer)
                        nc.sync.dma_start(
                            loc_p[:, bass.ds(iv0, unroll), :], stag[:])

                def kc_load(src_loc):
                    src_p = src_loc.rearrange("s p (c b) -> p s c b", c=KC)
                    for kc in range(KC):
                        nc.sync.dma_start(
                            kcs[kc][:], src_p[:, :, kc, :])

                def p1a_chunk(k):
                    # p1aT[j, (s,b)] = wih1^T-blocks . h0kc  (+ b1)
                    dst = p1aloc[k % 2].rearrange("s p (c b) -> p s c b", c=KC)
                    NT = CH * BL2 // 512  # N-tiles of 512
                    for jc in range(KC):
                        for nt in range(NT):
                            psb = prbig.tile([128, 512], F32, tag="psb")
                            for kc in range(KC):
                                nc.tensor.matmul(
                                    psb[:],
                                    w_block(wih1, kc, jc),
                                    kcs[kc][:, nt * 512:(nt + 1) * 512],
                                    start=(kc == 0), stop=(kc == KC - 1))
                            ev = prout.tile([128, 512], F32, tag="ev")
                            nc.scalar.activation(
                                ev[:], psb[:], AF.Identity,
                                bias=b1t[:, jc:jc + 1])
                            t0 = nt * (512 // BL2)
                            nc.sync.dma_start(
                                dst[:, t0:t0 + 512 // BL2, jc, :], ev[:])

                def proj_chunk(k):
                    y_p = y_d.rearrange("b s o -> s b o")
                    NM = CH * BL2 // 128  # m-tiles
                    for m in range(NM):
                        psy = prbig.tile([128, 512], F32, tag="psy")
                        nc.tensor.matmul(psy[:], ident[:], boutbc[:],
                                         start=True, stop=False)
                        for kc in range(KC):
                            nc.tensor.matmul(
                                psy[:], kcs[kc][:, m * 128:(m + 1) * 128],
                                wout[:, kc * O:(kc + 1) * O],
                                start=False, stop=(kc == KC - 1))
                        ysb = prout.tile([128, 512], F32, tag="ysb")
                        nc.vector.tensor_copy(ysb[:], psy[:])
                        t0 = k * CH + m * (128 // BL2)
                        nc.sync.dma_start(
                            y_p[t0:t0 + 128 // BL2, :, :], ysb[:])

                def one_rep():
                    for k in range(nch):
                        with tc.If(pid < 4) as _c:
                            recur_chunk(k, 0, h0loc[k % 2])
                            kc_load(h0loc[k % 2])
                            p1a_chunk(k)
                        nc.gpsimd.collective_compute(
                            "AllGather", mybir.AluOpType.bypass,
                            ins=[p1aloc[k % 2]],
                            outs=[p1adst[k % 2]],
                            replica_groups=groups,
                        )
                        with tc.If(pid > 3) as _c2:
                            recur_chunk(k, 1, h1loc[k % 2])
                            kc_load(h1loc[k % 2])
                            proj_chunk(k)

                one_rep()
                for _ in range(reps - 1):
                    nc.vector.memset(h0a[:], 0.0)
                    nc.vector.memset(h1a[:], 0.0)
                    one_rep()

    nc.compile()
    return nc


def _host_inputs_v2(x, w_ih, w_hh, b_ih, b_hh, w_out, b_out, s_steps=S,
                    recur_bf16=True):
    import ml_dtypes
    wd = ml_dtypes.bfloat16 if recur_bf16 else np.float32
    b0 = (b_ih[0] + b_hh[0]).astype(np.float32)
    b1 = (b_ih[1] + b_hh[1]).astype(np.float32)
    common = {
        "wih0t": _wtile(np.ascontiguousarray(w_ih[0].T)),
        "whh0t": _wtile(np.ascontiguousarray(w_hh[0].T)).astype(wd),
        "wih1t": _wtile(np.ascontiguousarray(w_ih[1].T)).astype(wd),
        "whh1t": _wtile(np.ascontiguousarray(w_hh[1].T)).astype(wd),
        "woutt": _wtile(np.ascontiguousarray(w_out.T)).astype(wd),
        "b0t": np.ascontiguousarray(b0.reshape(KC, 128).T),
        "b1t": np.ascontiguousarray(b1.reshape(KC, 128).T),
        "boutbc": np.ascontiguousarray(
            np.broadcast_to(b_out.astype(np.float32), (128, O))),
        "ident": np.eye(128, dtype=np.float32),
    }
    zeros_x = np.zeros((BL2, s_steps, D), np.float32)
    in_maps = []
    for c in range(N_CORES):
        m = dict(common)
        if c < 4:
            m["x"] = np.ascontiguousarray(
                x[c * BL2:(c + 1) * BL2, :s_steps].astype(np.float32))
        else:
            m["x"] = zeros_x
        in_maps.append(m)
    return in_maps


def _host_inputs(x, w_ih, w_hh, b_ih, b_hh, w_out, b_out, s_steps=S,
                 recur_bf16=False):
    """Build per-core in_maps from full inputs."""
    import ml_dtypes
    wd = ml_dtypes.bfloat16 if recur_bf16 else np.float32
    b0 = (b_ih[0] + b_hh[0]).astype(np.float32)
    b1 = (b_ih[1] + b_hh[1]).astype(np.float32)
    common = {
        "wih0t": _wtile(np.ascontiguousarray(w_ih[0].T)),
        "whh0t": _wtile(np.ascontiguousarray(w_hh[0].T)).astype(wd),
        "wih1t": _wtile(np.ascontiguousarray(w_ih[1].T)).astype(wd),
        "whh1t": _wtile(np.ascontiguousarray(w_hh[1].T)).astype(wd),
        "woutt": _wtile(np.ascontiguousarray(w_out.T)),
        "b0t": np.ascontiguousarray(b0.reshape(KC, 128).T),
        "b1bc": np.ascontiguousarray(
            np.repeat(b1.reshape(KC, 128).T, BL, axis=1)),
        "boutbc": np.ascontiguousarray(
            np.broadcast_to(b_out.astype(np.float32), (128, O))),
        "ident": np.eye(128, dtype=np.float32),
    }
    in_maps = []
    for c in range(N_CORES):
        m = dict(common)
        m["x"] = np.ascontiguousarray(
            x[c * BL:(c + 1) * BL, :s_steps].astype(np.float32))
        in_maps.append(m)
    return in_maps


_NC_CACHE = {}
USE_BF16 = True


def kernel(x, hidden, w_ih, w_hh, b_ih, b_hh, w_out, b_out):
    x = np.asarray(x, dtype=np.float32)
    key = ("v1", S, 8, USE_BF16)
    if key not in _NC_CACHE:
        _NC_CACHE[key] = build_kernel(S, unroll=8, static_loop=False, reps=1,
                                      recur_bf16=USE_BF16)
    nc = _NC_CACHE[key]
    in_maps = _host_inputs(x, np.asarray(w_ih), np.asarray(w_hh),
                           np.asarray(b_ih), np.asarray(b_hh),
                           np.asarray(w_out), np.asarray(b_out),
                           recur_bf16=USE_BF16)
    res = bass_utils.run_bass_kernel_spmd(
        nc, in_maps, core_ids=list(range(N_CORES)))
    y = np.concatenate([res.results[c]["y"] for c in range(N_CORES)], axis=0)
    return y.astype(np.float32)
